# revision 19
# baseline (speedup 1.0000x reference)
"""Trainium2 Bass kernel for the capsule-routing layer (nn_Caps_Layer).

Computation (per batch b of x [B, S, D], W [D, 25]):
  u_hat = (x_b @ W).reshape(S, 5, 5)           # [S, n, k], col = n*5+k
  b0 = 0;  for 4 routing iters:
    c = softmax_n(b)                            # over the 5 capsules
    v[n,k] = sum_s c[n,s] u_hat[s,n,k]
    out = v / sqrt(sum_k v^2 + 1e-7)
    b[n,s] = sum_k out[n,k] u_hat[s,n,k]
Returns out [B, 5, 5].

Sharding: pure data-parallel over batch across 8 NeuronCores (16 batches
each); W replicated; no collectives.

v3 design:
  - x and W cast to FLOAT16 on the host (fp16, not bf16: the routing
    iteration chaotically amplifies u_hat perturbations; bf16's 2^-8
    ulp costs ~1.6e-2 rel error while fp16 keeps it ~4e-3) — halves HBM
    traffic and keeps the XBAR DMA-transpose (2-byte only) usable.
  - xT arrives via XBAR DMA-transpose (HBM -> SBUF), eliminating every
    PE transpose and staging copy of x. Transposes alternate between
    the two HWDGE queues (sync / scalar).
  - u_hat natural layout straight out of the PE: per (s-chunk, d-block)
    matmul with the xT block STATIONARY and the tiny W block MOVING,
    accumulating over d-blocks in PSUM.
  - Routing free-axis layout (k, b, sh, n): broadcast multiplies keep a
    packed 2-byte last dim (2x DVE) and k-sums become paired adds.
  - One ones-matmul per iteration (moving = whole t tile, <=512 cols)
    plus a DVE sh-reduce replaces 4 serial accumulating matmuls; v then
    lives in SBUF f32 (no extra copy, exact squares).
  - Routing state v / rnrm / w_t kept in f32 (f32r for PE broadcasts):
    per-iteration rounding noise injections were the accuracy killers.
  - 1/sqrt = exp(-0.5*ln(.)): Exp and Ln share one ACT table with Copy/
    Square, so the activation table never reloads.
  - softmax normalize via a single divide op.
  - Routing for two groups is emitted op-interleaved (generators), so
    the in-order engines ping-pong between two independent dependency
    chains instead of idling on one; the first pair hides under the
    second pair's phase 1.
"""

from contextlib import ExitStack

import numpy as np

import concourse.bass as bass
import concourse.tile as tile
from concourse import mybir

F32 = mybir.dt.float32
F32R = mybir.dt.float32r
F16 = mybir.dt.float16
AX = mybir.AxisListType
OP = mybir.AluOpType
AF = mybir.ActivationFunctionType

N_CORES = 8
B_FULL, S, D = 128, 512, 768
NCAP, KDIM = 5, 5
NK = NCAP * KDIM  # 25
ROUTINGS = 4
T_EPS = 1e-7

ND = D // 128   # 6 d-blocks
NSB = S // 128  # 4 s-blocks


def emit(ctx, tc, out, x, w, b_loc=16, groups=(4, 4, 4, 4), dual_q=True):
    nc = tc.nc
    groups = list(groups)
    ngr = len(groups)
    assert sum(groups) == b_loc
    assert all(KDIM * g * NSB * NCAP <= 512 for g in groups), "pv matmul >512 cols"

    const_pool = ctx.enter_context(tc.tile_pool(name="const", bufs=1))
    xt_pool = ctx.enter_context(tc.tile_pool(name="xt", bufs=b_loc))
    pu_pool = ctx.enter_context(tc.tile_pool(name="pu", bufs=2, space="PSUM"))
    uh_pool = ctx.enter_context(tc.tile_pool(name="uh", bufs=2))
    rt_pool = ctx.enter_context(tc.tile_pool(name="rt", bufs=2))
    pv_pool = ctx.enter_context(tc.tile_pool(name="pv", bufs=2, space="PSUM"))

    # --- x transpose-DMAs first: the sync queue must not sit behind
    # anything (the XBAR stream is the pacing resource) ---
    xts = []
    for b in range(b_loc):
        xt = xt_pool.tile([128, ND * S], F16, tag="xt", name=f"xt{b}")
        eng = nc.sync if (b % 2 == 0 or not dual_q) else nc.scalar
        eng.dma_start(
            xt[:].rearrange("p (db s) -> p db s", db=ND),
            x[b * S:(b + 1) * S, :],
            transpose=True,
        )
        xts.append(xt)

    # --- constants (W is host-prearranged to [128, (db, nk)]: one clean
    # 300B-per-partition DMA on the scalar HWDGE queue) ---
    w_sb = const_pool.tile([128, ND * NK], F16)
    nc.scalar.dma_start(w_sb[:], w[:, :])
    ones_col = const_pool.tile([128, 1], F16)
    nc.gpsimd.memset(ones_col[:], 1.0)
    ones_row = const_pool.tile([1, 128], F16)
    nc.gpsimd.memset(ones_row[:], 1.0)
    eps1 = const_pool.tile([1, 1], F32)
    nc.gpsimd.memset(eps1[:], T_EPS)
    # iteration-0 softmax is uniform: fold c = 1/NCAP into the rsqrt
    # broadcast via exp(-0.5*ln(..) + ln(1/NCAP))
    lcs1 = const_pool.tile([1, 1], F32)
    nc.gpsimd.memset(lcs1[:], float(np.log(1.0 / NCAP)))
    zero1 = const_pool.tile([1, 1], F32)
    nc.gpsimd.memset(zero1[:], 0.0)
    outs_all = const_pool.tile([1, b_loc * NK], F32)

    def warm_pe():
        """Tiny fp16 matmul: keeps the HAM clock gate and PE p-state up
        through routing stretches where the PE would otherwise idle."""
        wps = pv_pool.tile([1, 64], F32, tag="warm", bufs=1)
        nc.tensor.matmul(wps[:], ones_col[:], w_sb[:, 0:64], start=True, stop=True)

    for _ in range(6):
        warm_pe()

    uh_tiles = {}

    def emit_phase1_batch(g, bi, b):
        """24 mini-matmuls (xT block stationary, W block moving) -> u_hat
        natural [128 s_lo, (sc, n, k)] in PSUM; copy into the group's uh
        tile in (k, b, sh, n) order (fp16)."""
        G = groups[g]
        if bi == 0:
            uh_tiles[g] = uh_pool.tile(
                [128, KDIM * G * NSB * NCAP], F16, tag="uh", name=f"uh{g}"
            )
        uh5 = uh_tiles[g][:].rearrange(
            "p (k b sh n) -> p k b sh n", k=KDIM, b=G, sh=NSB
        )
        xt3 = xts[b][:].rearrange("p (db s) -> p db s", db=ND)
        pu = pu_pool.tile([128, NSB * NK], F32, tag="pu")
        for sc in range(NSB):
            for db in range(ND):
                nc.tensor.matmul(
                    pu[:, sc * NK:(sc + 1) * NK],
                    xt3[:, db, sc * 128:(sc + 1) * 128],
                    w_sb[:, db * NK:(db + 1) * NK],
                    start=(db == 0),
                    stop=(db == ND - 1),
                )
        nc.scalar.copy(
            uh5[:, :, bi, :, :],
            pu[:].rearrange("p (sh n k) -> p k sh n", sh=NSB, n=NCAP),
        )

    blogs = {}

    def routing_iter_steps(g, it, b_off, chain_mode):
        """Generator emitting one routing iteration for group g, yielding
        between steps so two groups' chains can be op-interleaved.

        Free-axis layouts: blog [*, (b, sh, n)] f32, uh/t/tmp
        [*, (k, b, sh, n)] fp16, v/pv [*, (k, b, n)] f32.
        chain_mode=True keeps every op on the fast engines (last pair);
        otherwise bulk k-sums go to the idle Pool engine.
        """
        G = groups[g]
        uh5 = uh_tiles[g][:].rearrange(
            "p (k b sh n) -> p k b sh n", k=KDIM, b=G, sh=NSB
        )
        nbsn = G * NSB * NCAP
        nkbn = KDIM * G * NCAP
        if it == 0:
            t_mv = uh_tiles[g][:]
        else:
            blog = blogs[g]
            expb = rt_pool.tile([128, nbsn], F32, tag="expb")
            nc.scalar.activation(expb[:], blog[:], AF.Exp)
            yield
            den = rt_pool.tile([128, G * NSB], F32, tag="den")
            nc.vector.reduce_sum(
                den[:],
                expb[:].rearrange("p (bs n) -> p bs n", n=NCAP),
                axis=AX.X,
            )
            yield
            rden = rt_pool.tile([128, G * NSB], F32, tag="rden")
            nc.vector.reciprocal(rden[:], den[:])
            yield
            c = rt_pool.tile([128, nbsn], F16, tag="c")
            nc.vector.tensor_tensor(
                c[:].rearrange("p (b sh n) -> p b sh n", b=G, sh=NSB),
                expb[:].rearrange("p (b sh n) -> p b sh n", b=G, sh=NSB),
                rden[:].rearrange("p (b sh) -> p b sh", b=G)
                .unsqueeze(3)
                .broadcast_to((128, G, NSB, NCAP)),
                op=OP.mult,
            )
            yield
            t = rt_pool.tile([128, KDIM * nbsn], F16, tag="t")
            nc.vector.tensor_tensor(
                t[:].rearrange("p (k b sh n) -> p k b sh n", k=KDIM, b=G, sh=NSB),
                uh5,
                c[:].rearrange("p (b sh n) -> p b sh n", b=G, sh=NSB)
                .unsqueeze(1)
                .broadcast_to((128, KDIM, G, NSB, NCAP)),
                op=OP.mult,
            )
            t_mv = t[:]
            yield
        # ---- raw v[(k, b, n)] = sum_s t: one ones-matmul (partition sum,
        # sh stays in the free axis) + DVE sh-reduce -> v in SBUF f32 ----
        pv = pv_pool.tile([1, KDIM * nbsn], F32, tag="pv")
        nc.tensor.matmul(pv[:], ones_col[:], t_mv, start=True, stop=True)
        warm_pe()
        yield
        v = rt_pool.tile([1, nkbn], F32, tag="v")
        nc.vector.reduce_sum(
            v[:],
            pv[:].rearrange("o (k b sh n) -> o k b n sh", k=KDIM, b=G, sh=NSB),
            axis=AX.X,
        )
        yield
        # ---- side branch: rnrm = exp(-0.5*ln(cs^2*|v|^2 + eps) [+ ln cs])
        # (Exp/Ln/Copy/Square share one ACT table: never reloads) ----
        cs = 1.0 / NCAP if it == 0 else 1.0
        sq = rt_pool.tile([1, nkbn], F32, tag="sq")
        nc.gpsimd.tensor_tensor(sq[:], v[:], v[:], op=OP.mult)
        yield
        s2 = rt_pool.tile([1, G * NCAP], F32, tag="s2")
        nc.vector.reduce_sum(
            s2[:],
            sq[:].rearrange("o (k b n) -> o b n k", k=KDIM, b=G),
            axis=AX.X,
        )
        yield
        lnv = rt_pool.tile([1, G * NCAP], F32, tag="lnv")
        nc.scalar.activation(lnv[:], s2[:], AF.Ln, bias=eps1[:], scale=cs * cs)
        yield
        if it < ROUTINGS - 1:
            rnrm = rt_pool.tile([1, G * NCAP], F16, tag="rnrm")
            nc.scalar.activation(
                rnrm[:], lnv[:], AF.Exp,
                bias=lcs1[:] if it == 0 else zero1[:], scale=-0.5,
            )
            yield
            # ---- main chain: w = sum_k v*u_hat, logits = w * rnrm ----
            v16 = rt_pool.tile([1, nkbn], F16, tag="v16")
            nc.scalar.copy(v16[:], v[:])
            yield
            pvb = pv_pool.tile([128, nkbn], F32, tag="pvb")
            nc.tensor.matmul(pvb[:], ones_row[:], v16[:], start=True, stop=True)
            yield
            pvb_sb = rt_pool.tile([128, nkbn], F16, tag="pvb_sb")
            nc.scalar.copy(pvb_sb[:], pvb[:])
            yield
            tmp = rt_pool.tile([128, KDIM * nbsn], F16, tag="tmp")
            nc.vector.tensor_tensor(
                tmp[:].rearrange(
                    "p (k b sh n) -> p k b sh n", k=KDIM, b=G, sh=NSB
                ),
                uh5,
                pvb_sb[:].rearrange("p (k b n) -> p k b n", k=KDIM, b=G)
                .unsqueeze(3)
                .broadcast_to((128, KDIM, G, NSB, NCAP)),
                op=OP.mult,
            )
            yield
            tk = [tmp[:, k * nbsn:(k + 1) * nbsn] for k in range(KDIM)]
            wa = rt_pool.tile([128, nbsn], F16, tag="wa")
            nc.gpsimd.tensor_tensor(wa[:], tk[0], tk[1], op=OP.add)
            wb = rt_pool.tile([128, nbsn], F16, tag="wb")
            nc.gpsimd.tensor_tensor(wb[:], tk[2], tk[3], op=OP.add)
            yield
            wc = rt_pool.tile([128, nbsn], F16, tag="wc")
            nc.vector.tensor_tensor(wc[:], wa[:], wb[:], op=OP.add)
            wt = rt_pool.tile([128, nbsn], F32, tag="wt")
            nc.vector.tensor_tensor(wt[:], wc[:], tk[4], op=OP.add)
            yield
            prn = pv_pool.tile([128, G * NCAP], F32, tag="prn", bufs=1)
            nc.tensor.matmul(prn[:], ones_row[:], rnrm[:], start=True, stop=True)
            yield
            blog = rt_pool.tile([128, nbsn], F32, tag="blog")
            nc.vector.tensor_tensor(
                blog[:].rearrange("p (b sh n) -> p b sh n", b=G, sh=NSB),
                wt[:].rearrange("p (b sh n) -> p b sh n", b=G, sh=NSB),
                prn[:].rearrange("p (b n) -> p b n", b=G)
                .unsqueeze(2)
                .broadcast_to((128, G, NSB, NCAP)),
                op=OP.mult,
            )
            blogs[g] = blog
            yield
        else:
            # ---- final outputs (cs == 1): v * rnrm into the gather tile ----
            rnrm_f = rt_pool.tile([1, G * NCAP], F32, tag="rnrm_f")
            nc.scalar.activation(rnrm_f[:], lnv[:], AF.Exp, scale=-0.5)
            yield
            nc.vector.tensor_tensor(
                outs_all[0:1, b_off * NK:(b_off + G) * NK]
                .rearrange("o (b n k) -> o k b n", n=NCAP, k=KDIM),
                v[:].rearrange("o (k b n) -> o k b n", k=KDIM, b=G),
                rnrm_f[:].rearrange("o (b n) -> o b n", b=G)
                .unsqueeze(1)
                .broadcast_to((1, KDIM, G, NCAP)),
                op=OP.mult,
            )
            yield

    def pair_steps(ga, gb, it, offs, chain_mode):
        """Op-interleave one iteration of two independent groups."""
        gens = [routing_iter_steps(ga, it, offs[ga], chain_mode)]
        if gb is not None:
            gens.append(routing_iter_steps(gb, it, offs[gb], chain_mode))
        alive = [True] * len(gens)
        while any(alive):
            for i, gen in enumerate(gens):
                if alive[i]:
                    try:
                        next(gen)
                    except StopIteration:
                        alive[i] = False

    # ---- schedule ----
    offs = [sum(groups[:i]) for i in range(ngr)]
    pairs = [(i, i + 1 if i + 1 < ngr else None) for i in range(0, ngr, 2)]

    # phase 1 of the first pair
    for g in pairs[0][:2]:
        if g is None:
            continue
        for bi in range(groups[g]):
            emit_phase1_batch(g, bi, offs[g] + bi)

    for pi, (ga, gb) in enumerate(pairs):
        last = pi + 1 >= len(pairs)
        # batches of the NEXT pair, to weave between this pair's iters
        nxt = []
        if not last:
            for g in pairs[pi + 1][:2]:
                if g is None:
                    continue
                nxt += [(g, bi, offs[g] + bi) for bi in range(groups[g])]
        per_iter = (len(nxt) + ROUTINGS - 1) // ROUTINGS if nxt else 0
        for it in range(ROUTINGS):
            for _ in range(per_iter):
                if nxt:
                    emit_phase1_batch(*nxt.pop(0))
            pair_steps(ga, gb, it, offs, chain_mode=last)
        while nxt:
            emit_phase1_batch(*nxt.pop(0))

    nc.sync.dma_start(out[0:1, :], outs_all[0:1, :])


def legalize_waits(nc):
    """This toolchain's walrus codegen accepts at most ONE sync wait per
    instruction ("Too many sync wait commands" otherwise) — and PE Matmult
    appears to take none safely. Hoist excess waits onto wait-only
    EventSemaphore instructions inserted just before, on the same engine
    (same pattern walrus already accepts for Tile's engine barriers)."""
    n = 0
    for fn in nc.m.functions:
        for blk in fn.blocks:
            new = []
            for inst in blk.instructions:
                si = inst.sync_info
                if si is not None and len(si.on_wait) > 0:
                    waits = list(si.on_wait)
                    keep = 0 if type(inst).__name__ == "InstMatmult" else 1
                    if len(waits) > keep:
                        for wt in waits[: len(waits) - keep]:
                            ev = mybir.InstEventSemaphore(
                                name=f"I-waitfix-{nc.next_id()}"
                            )
                            ev.engine = inst.engine
                            ev.sync_info = mybir.SyncInfo(on_wait=[wt], on_update=[])
                            new.append(ev)
                            n += 1
                        si.on_wait = waits[len(waits) - keep:]
                new.append(inst)
            blk.instructions = new
    return n


def build_caps_kernel(b_loc=16, groups=(4, 4, 4, 4), dual_q=True):
    nc = bass.Bass(trn_type="TRN2", debug=False, target_bir_lowering=False)
    x = nc.dram_tensor("x", [b_loc * S, D], F16, kind="ExternalInput").ap()
    w = nc.dram_tensor("w", [128, ND * NK], F16, kind="ExternalInput").ap()
    out = nc.dram_tensor("out", [1, b_loc * NK], F32, kind="ExternalOutput").ap()
    with nc.allow_low_precision(reason="fp16 k-sums; f32 state"):
        with tile.TileContext(nc) as tc:
            with ExitStack() as ctx:
                emit(ctx, tc, out, x, w, b_loc=b_loc, groups=groups, dual_q=dual_q)
    legalize_waits(nc)
    return nc


# dual_q=True races concurrent XBAR transposes from the two HWDGE queues
# and corrupts the loads (measured rel err ~1.4) — keep single-queue.
_KERNEL_CFG = dict(groups=(5, 5, 4, 2), dual_q=False)


def make_inmaps(x: np.ndarray, W: np.ndarray, b_loc: int):
    xh = x.astype(np.float16)
    # pre-arrange W to the on-chip layout [128 d_lo, (d_block, nk)]
    wh = np.ascontiguousarray(
        W.reshape(ND, 128, NK).transpose(1, 0, 2).reshape(128, ND * NK)
        .astype(np.float16)
    )
    return [
        {
            "x": np.ascontiguousarray(
                xh[i * b_loc:(i + 1) * b_loc].reshape(b_loc * S, D)
            ),
            "w": wh,
        }
        for i in range(N_CORES)
    ]


def kernel(x: np.ndarray, W: np.ndarray) -> np.ndarray:
    from concourse.bass_utils import run_bass_kernel_spmd

    B, S_, D_ = x.shape
    assert (B, S_, D_) == (B_FULL, S, D)
    b_loc = B // N_CORES
    nc = build_caps_kernel(b_loc=b_loc, **_KERNEL_CFG)
    in_maps = make_inmaps(x, W, b_loc)
    res = run_bass_kernel_spmd(nc, in_maps, core_ids=list(range(N_CORES)))
    outs = [res.results[i]["out"].reshape(b_loc, NCAP, KDIM) for i in range(N_CORES)]
    return np.concatenate(outs, axis=0).astype(np.float32)


# revision 20
# speedup vs baseline: 1.1949x; 1.1949x over previous
"""Trainium2 Bass kernel for the capsule-routing layer (nn_Caps_Layer).

Computation (per batch b of x [B, S, D], W [D, 25]):
  u_hat = (x_b @ W).reshape(S, 5, 5)           # [S, n, k], col = n*5+k
  b0 = 0;  for 4 routing iters:
    c = softmax_n(b)                            # over the 5 capsules
    v[n,k] = sum_s c[n,s] u_hat[s,n,k]
    out = v / sqrt(sum_k v^2 + 1e-7)
    b[n,s] = sum_k out[n,k] u_hat[s,n,k]
Returns out [B, 5, 5].

Sharding: pure data-parallel over batch across 8 NeuronCores (16 batches
each); W replicated; no collectives.

v3 design:
  - x and W cast to FLOAT16 on the host (fp16, not bf16: the routing
    iteration chaotically amplifies u_hat perturbations; bf16's 2^-8
    ulp costs ~1.6e-2 rel error while fp16 keeps it ~4e-3) — halves HBM
    traffic and keeps the XBAR DMA-transpose (2-byte only) usable.
  - xT arrives via XBAR DMA-transpose (HBM -> SBUF), eliminating every
    PE transpose and staging copy of x. Transposes alternate between
    the two HWDGE queues (sync / scalar).
  - u_hat natural layout straight out of the PE: per (s-chunk, d-block)
    matmul with the xT block STATIONARY and the tiny W block MOVING,
    accumulating over d-blocks in PSUM.
  - Routing free-axis layout (k, b, sh, n): broadcast multiplies keep a
    packed 2-byte last dim (2x DVE) and k-sums become paired adds.
  - One ones-matmul per iteration (moving = whole t tile, <=512 cols)
    plus a DVE sh-reduce replaces 4 serial accumulating matmuls; v then
    lives in SBUF f32 (no extra copy, exact squares).
  - Routing state v / rnrm / w_t kept in f32 (f32r for PE broadcasts):
    per-iteration rounding noise injections were the accuracy killers.
  - 1/sqrt = exp(-0.5*ln(.)): Exp and Ln share one ACT table with Copy/
    Square, so the activation table never reloads.
  - softmax normalize via a single divide op.
  - Routing for two groups is emitted op-interleaved (generators), so
    the in-order engines ping-pong between two independent dependency
    chains instead of idling on one; the first pair hides under the
    second pair's phase 1.
"""

from contextlib import ExitStack

import numpy as np

import concourse.bass as bass
import concourse.tile as tile
from concourse import mybir

F32 = mybir.dt.float32
F32R = mybir.dt.float32r
F16 = mybir.dt.float16
AX = mybir.AxisListType
OP = mybir.AluOpType
AF = mybir.ActivationFunctionType

N_CORES = 8
B_FULL, S, D = 128, 512, 768
NCAP, KDIM = 5, 5
NK = NCAP * KDIM  # 25
ROUTINGS = 4
T_EPS = 1e-7

ND = D // 128   # 6 d-blocks
NSB = S // 128  # 4 s-blocks


def emit(ctx, tc, out, x, w, b_loc=16, groups=(4, 4, 4, 4), dual_q=True):
    nc = tc.nc
    groups = list(groups)
    ngr = len(groups)
    assert sum(groups) == b_loc
    assert all(KDIM * g * NSB * NCAP <= 512 for g in groups), "pv matmul >512 cols"

    const_pool = ctx.enter_context(tc.tile_pool(name="const", bufs=1))
    xt_pool = ctx.enter_context(tc.tile_pool(name="xt", bufs=b_loc))
    pu_pool = ctx.enter_context(tc.tile_pool(name="pu", bufs=2, space="PSUM"))
    uh_pool = ctx.enter_context(tc.tile_pool(name="uh", bufs=2))
    rt_pool = ctx.enter_context(tc.tile_pool(name="rt", bufs=2))
    pv_pool = ctx.enter_context(tc.tile_pool(name="pv", bufs=2, space="PSUM"))

    # --- x transpose-DMAs first: the sync queue must not sit behind
    # anything (the XBAR stream is the pacing resource) ---
    xts = []
    for b in range(b_loc):
        xt = xt_pool.tile([128, ND * S], F16, tag="xt", name=f"xt{b}")
        eng = nc.sync if (b % 2 == 0 or not dual_q) else nc.scalar
        eng.dma_start(
            xt[:].rearrange("p (db s) -> p db s", db=ND),
            x[b * S:(b + 1) * S, :],
            transpose=True,
        )
        xts.append(xt)

    # --- constants (W is host-prearranged to [128, (db, nk)]: one clean
    # 300B-per-partition DMA on the scalar HWDGE queue) ---
    w_sb = const_pool.tile([128, ND * NK], F16)
    nc.scalar.dma_start(w_sb[:], w[:, :])
    ones_col = const_pool.tile([128, 1], F16)
    nc.gpsimd.memset(ones_col[:], 1.0)
    ones_row = const_pool.tile([1, 128], F16)
    nc.gpsimd.memset(ones_row[:], 1.0)
    eps1 = const_pool.tile([1, 1], F32)
    nc.gpsimd.memset(eps1[:], T_EPS)
    # iteration-0 softmax is uniform: fold c = 1/NCAP into the rsqrt
    # broadcast via exp(-0.5*ln(..) + ln(1/NCAP))
    lcs1 = const_pool.tile([1, 1], F32)
    nc.gpsimd.memset(lcs1[:], float(np.log(1.0 / NCAP)))
    zero1 = const_pool.tile([1, 1], F32)
    nc.gpsimd.memset(zero1[:], 0.0)
    outs_all = const_pool.tile([1, b_loc * NK], F32)

    def warm_pe():
        """Tiny fp16 matmul: keeps the HAM clock gate and PE p-state up
        through routing stretches where the PE would otherwise idle."""
        wps = pv_pool.tile([1, 64], F32, tag="warm", bufs=1)
        nc.tensor.matmul(wps[:], ones_col[:], w_sb[:, 0:64], start=True, stop=True)

    for _ in range(6):
        warm_pe()

    uh_tiles = {}

    def emit_phase1_batch(g, bi, b):
        """24 mini-matmuls (xT block stationary, W block moving) -> u_hat
        natural [128 s_lo, (sc, n, k)] in PSUM; copy into the group's uh
        tile in (k, b, sh, n) order (fp16)."""
        G = groups[g]
        if bi == 0:
            uh_tiles[g] = uh_pool.tile(
                [128, KDIM * G * NSB * NCAP], F16, tag="uh", name=f"uh{g}"
            )
        uh5 = uh_tiles[g][:].rearrange(
            "p (k b sh n) -> p k b sh n", k=KDIM, b=G, sh=NSB
        )
        xt3 = xts[b][:].rearrange("p (db s) -> p db s", db=ND)
        pu = pu_pool.tile([128, NSB * NK], F32, tag="pu")
        for sc in range(NSB):
            for db in range(ND):
                nc.tensor.matmul(
                    pu[:, sc * NK:(sc + 1) * NK],
                    xt3[:, db, sc * 128:(sc + 1) * 128],
                    w_sb[:, db * NK:(db + 1) * NK],
                    start=(db == 0),
                    stop=(db == ND - 1),
                )
        nc.scalar.copy(
            uh5[:, :, bi, :, :],
            pu[:].rearrange("p (sh n k) -> p k sh n", sh=NSB, n=NCAP),
        )

    blogs = {}

    def routing_iter_steps(g, it, b_off, chain_mode):
        """Generator emitting one routing iteration for group g, yielding
        between steps so two groups' chains can be op-interleaved.

        Free-axis layouts: blog [*, (b, sh, n)] f32, uh/t/tmp
        [*, (k, b, sh, n)] fp16, v/pv [*, (k, b, n)] f32.
        chain_mode=True keeps every op on the fast engines (last pair);
        otherwise bulk k-sums go to the idle Pool engine.
        """
        G = groups[g]
        uh5 = uh_tiles[g][:].rearrange(
            "p (k b sh n) -> p k b sh n", k=KDIM, b=G, sh=NSB
        )
        nbsn = G * NSB * NCAP
        nkbn = KDIM * G * NCAP
        if it == 0:
            t_mv = uh_tiles[g][:]
        else:
            blog = blogs[g]
            expb = rt_pool.tile([128, nbsn], F32, tag="expb")
            nc.scalar.activation(expb[:], blog[:], AF.Exp)
            yield
            den = rt_pool.tile([128, G * NSB], F32, tag="den")
            nc.vector.reduce_sum(
                den[:],
                expb[:].rearrange("p (bs n) -> p bs n", n=NCAP),
                axis=AX.X,
            )
            yield
            rden = rt_pool.tile([128, G * NSB], F32, tag="rden")
            nc.vector.reciprocal(rden[:], den[:])
            yield
            c = rt_pool.tile([128, nbsn], F16, tag="c")
            nc.vector.tensor_tensor(
                c[:].rearrange("p (b sh n) -> p b sh n", b=G, sh=NSB),
                expb[:].rearrange("p (b sh n) -> p b sh n", b=G, sh=NSB),
                rden[:].rearrange("p (b sh) -> p b sh", b=G)
                .unsqueeze(3)
                .broadcast_to((128, G, NSB, NCAP)),
                op=OP.mult,
            )
            yield
            t = rt_pool.tile([128, KDIM * nbsn], F16, tag="t")
            nc.vector.tensor_tensor(
                t[:].rearrange("p (k b sh n) -> p k b sh n", k=KDIM, b=G, sh=NSB),
                uh5,
                c[:].rearrange("p (b sh n) -> p b sh n", b=G, sh=NSB)
                .unsqueeze(1)
                .broadcast_to((128, KDIM, G, NSB, NCAP)),
                op=OP.mult,
            )
            t_mv = t[:]
            yield
        # ---- raw v[(k, b, n)] = sum_s t: one ones-matmul (partition sum,
        # sh stays in the free axis) + DVE sh-reduce -> v in SBUF f32 ----
        pv = pv_pool.tile([1, KDIM * nbsn], F32, tag="pv")
        nc.tensor.matmul(pv[:], ones_col[:], t_mv, start=True, stop=True)
        warm_pe()
        yield
        v = rt_pool.tile([1, nkbn], F32, tag="v")
        nc.vector.reduce_sum(
            v[:],
            pv[:].rearrange("o (k b sh n) -> o k b n sh", k=KDIM, b=G, sh=NSB),
            axis=AX.X,
        )
        yield
        # ---- side branch: rnrm = exp(-0.5*ln(cs^2*|v|^2 + eps) [+ ln cs])
        # (Exp/Ln/Copy/Square share one ACT table: never reloads) ----
        cs = 1.0 / NCAP if it == 0 else 1.0
        sq = rt_pool.tile([1, nkbn], F32, tag="sq")
        nc.gpsimd.tensor_tensor(sq[:], v[:], v[:], op=OP.mult)
        yield
        s2 = rt_pool.tile([1, G * NCAP], F32, tag="s2")
        nc.vector.reduce_sum(
            s2[:],
            sq[:].rearrange("o (k b n) -> o b n k", k=KDIM, b=G),
            axis=AX.X,
        )
        yield
        lnv = rt_pool.tile([1, G * NCAP], F32, tag="lnv")
        nc.scalar.activation(lnv[:], s2[:], AF.Ln, bias=eps1[:], scale=cs * cs)
        yield
        if it < ROUTINGS - 1:
            rnrm = rt_pool.tile([1, G * NCAP], F16, tag="rnrm")
            nc.scalar.activation(
                rnrm[:], lnv[:], AF.Exp,
                bias=lcs1[:] if it == 0 else zero1[:], scale=-0.5,
            )
            yield
            # ---- main chain: w = sum_k v*u_hat, logits = w * rnrm ----
            v16 = rt_pool.tile([1, nkbn], F16, tag="v16")
            nc.scalar.copy(v16[:], v[:])
            yield
            pvb = pv_pool.tile([128, nkbn], F32, tag="pvb")
            nc.tensor.matmul(pvb[:], ones_row[:], v16[:], start=True, stop=True)
            yield
            pvb_sb = rt_pool.tile([128, nkbn], F16, tag="pvb_sb")
            nc.scalar.copy(pvb_sb[:], pvb[:])
            yield
            tmp = rt_pool.tile([128, KDIM * nbsn], F16, tag="tmp")
            nc.vector.tensor_tensor(
                tmp[:].rearrange(
                    "p (k b sh n) -> p k b sh n", k=KDIM, b=G, sh=NSB
                ),
                uh5,
                pvb_sb[:].rearrange("p (k b n) -> p k b n", k=KDIM, b=G)
                .unsqueeze(3)
                .broadcast_to((128, KDIM, G, NSB, NCAP)),
                op=OP.mult,
            )
            yield
            tk = [tmp[:, k * nbsn:(k + 1) * nbsn] for k in range(KDIM)]
            wa = rt_pool.tile([128, nbsn], F16, tag="wa")
            nc.gpsimd.tensor_tensor(wa[:], tk[0], tk[1], op=OP.add)
            wb = rt_pool.tile([128, nbsn], F16, tag="wb")
            nc.gpsimd.tensor_tensor(wb[:], tk[2], tk[3], op=OP.add)
            yield
            wc = rt_pool.tile([128, nbsn], F16, tag="wc")
            nc.vector.tensor_tensor(wc[:], wa[:], wb[:], op=OP.add)
            wt = rt_pool.tile([128, nbsn], F32, tag="wt")
            nc.vector.tensor_tensor(wt[:], wc[:], tk[4], op=OP.add)
            yield
            prn = pv_pool.tile([128, G * NCAP], F32, tag="prn", bufs=1)
            nc.tensor.matmul(prn[:], ones_row[:], rnrm[:], start=True, stop=True)
            yield
            blog = rt_pool.tile([128, nbsn], F32, tag="blog")
            nc.vector.tensor_tensor(
                blog[:].rearrange("p (b sh n) -> p b sh n", b=G, sh=NSB),
                wt[:].rearrange("p (b sh n) -> p b sh n", b=G, sh=NSB),
                prn[:].rearrange("p (b n) -> p b n", b=G)
                .unsqueeze(2)
                .broadcast_to((128, G, NSB, NCAP)),
                op=OP.mult,
            )
            blogs[g] = blog
            yield
        else:
            # ---- final outputs (cs == 1): v * rnrm into the gather tile ----
            rnrm_f = rt_pool.tile([1, G * NCAP], F32, tag="rnrm_f")
            nc.scalar.activation(rnrm_f[:], lnv[:], AF.Exp, scale=-0.5)
            yield
            nc.vector.tensor_tensor(
                outs_all[0:1, b_off * NK:(b_off + G) * NK]
                .rearrange("o (b n k) -> o k b n", n=NCAP, k=KDIM),
                v[:].rearrange("o (k b n) -> o k b n", k=KDIM, b=G),
                rnrm_f[:].rearrange("o (b n) -> o b n", b=G)
                .unsqueeze(1)
                .broadcast_to((1, KDIM, G, NCAP)),
                op=OP.mult,
            )
            yield

    def pair_steps(ga, gb, it, offs, chain_mode):
        """Op-interleave one iteration of two independent groups."""
        gens = [routing_iter_steps(ga, it, offs[ga], chain_mode)]
        if gb is not None:
            gens.append(routing_iter_steps(gb, it, offs[gb], chain_mode))
        alive = [True] * len(gens)
        while any(alive):
            for i, gen in enumerate(gens):
                if alive[i]:
                    try:
                        next(gen)
                    except StopIteration:
                        alive[i] = False

    # ---- schedule ----
    offs = [sum(groups[:i]) for i in range(ngr)]
    pairs = [(i, i + 1 if i + 1 < ngr else None) for i in range(0, ngr, 2)]

    # phase 1 of the first pair
    for g in pairs[0][:2]:
        if g is None:
            continue
        for bi in range(groups[g]):
            emit_phase1_batch(g, bi, offs[g] + bi)

    for pi, (ga, gb) in enumerate(pairs):
        last = pi + 1 >= len(pairs)
        # batches of the NEXT pair, to weave between this pair's iters
        nxt = []
        if not last:
            for g in pairs[pi + 1][:2]:
                if g is None:
                    continue
                nxt += [(g, bi, offs[g] + bi) for bi in range(groups[g])]
        per_iter = (len(nxt) + ROUTINGS - 1) // ROUTINGS if nxt else 0
        for it in range(ROUTINGS):
            for _ in range(per_iter):
                if nxt:
                    emit_phase1_batch(*nxt.pop(0))
            pair_steps(ga, gb, it, offs, chain_mode=last)
        while nxt:
            emit_phase1_batch(*nxt.pop(0))

    nc.sync.dma_start(out[0:1, :], outs_all[0:1, :])


def legalize_waits(nc):
    """This toolchain's walrus codegen accepts at most ONE sync wait per
    instruction ("Too many sync wait commands" otherwise) — and PE Matmult
    appears to take none safely. Hoist excess waits onto wait-only
    EventSemaphore instructions inserted just before, on the same engine
    (same pattern walrus already accepts for Tile's engine barriers)."""
    n = 0
    for fn in nc.m.functions:
        for blk in fn.blocks:
            new = []
            for inst in blk.instructions:
                si = inst.sync_info
                if si is not None and len(si.on_wait) > 0:
                    waits = list(si.on_wait)
                    keep = 0 if type(inst).__name__ == "InstMatmult" else 1
                    if len(waits) > keep:
                        for wt in waits[: len(waits) - keep]:
                            ev = mybir.InstEventSemaphore(
                                name=f"I-waitfix-{nc.next_id()}"
                            )
                            ev.engine = inst.engine
                            ev.sync_info = mybir.SyncInfo(on_wait=[wt], on_update=[])
                            new.append(ev)
                            n += 1
                        si.on_wait = waits[len(waits) - keep:]
                new.append(inst)
            blk.instructions = new
    return n


def build_caps_kernel(b_loc=16, groups=(4, 4, 4, 4), dual_q=True):
    nc = bass.Bass(trn_type="TRN2", debug=False, target_bir_lowering=False)
    x = nc.dram_tensor("x", [b_loc * S, D], F16, kind="ExternalInput").ap()
    w = nc.dram_tensor("w", [128, ND * NK], F16, kind="ExternalInput").ap()
    out = nc.dram_tensor("out", [1, b_loc * NK], F32, kind="ExternalOutput").ap()
    with nc.allow_low_precision(reason="fp16 k-sums; f32 state"):
        with tile.TileContext(nc) as tc:
            with ExitStack() as ctx:
                emit(ctx, tc, out, x, w, b_loc=b_loc, groups=groups, dual_q=dual_q)
    legalize_waits(nc)
    return nc


# dual_q=True races concurrent XBAR transposes from the two HWDGE queues
# and corrupts the loads (measured rel err ~1.4) — keep single-queue.
_KERNEL_CFG = dict(groups=(4, 4, 4, 4), dual_q=False)


def make_inmaps(x: np.ndarray, W: np.ndarray, b_loc: int):
    xh = x.astype(np.float16)
    # pre-arrange W to the on-chip layout [128 d_lo, (d_block, nk)]
    wh = np.ascontiguousarray(
        W.reshape(ND, 128, NK).transpose(1, 0, 2).reshape(128, ND * NK)
        .astype(np.float16)
    )
    return [
        {
            "x": np.ascontiguousarray(
                xh[i * b_loc:(i + 1) * b_loc].reshape(b_loc * S, D)
            ),
            "w": wh,
        }
        for i in range(N_CORES)
    ]


def kernel(x: np.ndarray, W: np.ndarray) -> np.ndarray:
    from concourse.bass_utils import run_bass_kernel_spmd

    B, S_, D_ = x.shape
    assert (B, S_, D_) == (B_FULL, S, D)
    b_loc = B // N_CORES
    nc = build_caps_kernel(b_loc=b_loc, **_KERNEL_CFG)
    in_maps = make_inmaps(x, W, b_loc)
    res = run_bass_kernel_spmd(nc, in_maps, core_ids=list(range(N_CORES)))
    outs = [res.results[i]["out"].reshape(b_loc, NCAP, KDIM) for i in range(N_CORES)]
    return np.concatenate(outs, axis=0).astype(np.float32)
